# revision 1
# baseline (speedup 1.0000x reference)
"""CustomGAT (gnn_message_passing) Trainium2 kernel — 8-core SPMD.

Strategy (dst-partitioned edge parallelism, zero output collectives):
  * Host: add self-loops, LPT-balance destination nodes into (8 cores x BPC
    blocks) of 128 slots each by in-degree, group edges by dst-block, pad
    each block's edge list to NCHUNK chunks of 128 edges. Fold attn_l/attn_r
    into the projection weights so one matmul emits [xp | al | ar] rows.
  * Device phase A (replicated on each core): projection table
    TabX[slots, 384] bf16 rows = [xp bf16 x256 | al f32 x8 | ar f32 x8 |
    pad] written to HBM scratch.
  * Device phase B (per dst-block): batched edge gathers via the GPSIMD
    dma_gather ucode (two passes, lo/hi row halves, negative int16 indices
    skip slots), attention logits from the packed f32 al/ar, leaky-relu +
    exp on ACT/DVE, scatter-add via one-hot matmuls accumulated in PSUM
    (the alpha ride-along column gives the softmax denominator), then a
    per-head normalize at block end.
  * Host: concatenate per-core output shards, inverse-permute slots.
"""

import math

import numpy as np

# ---------------------------------------------------------------- constants
H = 8
C = 32
HC = H * C  # 256
IN = 256
ROW = 384  # bf16 slots: [xp 0:256 | al f32 256:272 | ar f32 272:288 | pad]
PSROW = HC + 2 * H  # 272 fp32 projection row [xp | al | ar]
P = 128
# dma_gather runtime offset (idx * row_bytes) tops out near 2^24 bytes;
# with 768B rows that caps idx at 21845. Split the table into ranges.
RNG = 21504

USE_F32R = True  # full-rate fp32 matmuls (tf32-like) for phase A


# ---------------------------------------------------------------- tile patch
def _install_tile_patch():
    """The axon-path walrus rejects >2 sync waits on one instruction; split
    the TileContext tail-drain waits into one carrier drain per proc."""
    import concourse.tile as tile
    from concourse.vector_clock import ScopedClock, VectorClock

    if getattr(tile.TileContext, "_drain_patch_installed", False):
        return

    def _drain_and_barrier(self, tick_clock, wait_clock):
        gc = tick_clock.global_clock
        n = len(gc)
        for p in range(n):
            if gc[p] == 0:
                continue
            req = VectorClock([gc[q] if q == p else 0 for q in range(n)])
            d = self.nc.sync.drain()
            wait_clock.add_sem_waits(d.ins, ScopedClock({None: req}))
        self.nc.all_engine_barrier()
        assert self.sems is not None
        popped = self.nc._tile_sem_poison_stack.pop()
        assert popped is self._sem_poison
        self.nc.clear_and_free_semaphores(list(self.sems.allocated().values()))
        self.nc.all_engine_barrier()

    tile.TileContext._drain_and_barrier = _drain_and_barrier
    tile.TileContext._drain_patch_installed = True


# ---------------------------------------------------------------- host prep
def _idx16(vals, nchunk, _unused=None):
    """Encode row indices for dma_gather: [128, nchunk*8] int16, index k at
    [k%16 (+16*rep), k//16]."""
    enc = vals.astype(np.int64).astype(np.int16)
    a = enc.reshape(nchunk * 8, 16).T  # [16, nchunk*8]
    return np.tile(a, (8, 1))  # replicate for the 8 Q7 cores


def _preprocess(x, edge_index, W, attn_l, attn_r, n_cores):
    N = x.shape[0]

    src = np.concatenate([np.asarray(edge_index[0]), np.arange(N, dtype=np.int64)])
    dst = np.concatenate([np.asarray(edge_index[1]), np.arange(N, dtype=np.int64)])
    Etot = src.shape[0]

    bpc = math.ceil(N / (n_cores * P))  # blocks per core
    nblocks = n_cores * bpc
    slots = nblocks * P

    # LPT balance: assign nodes to blocks by descending in-degree.
    deg = np.bincount(dst, minlength=N).astype(np.int64)
    order = np.argsort(-deg, kind="stable")
    import heapq

    heap = [(0, b) for b in range(nblocks)]
    heapq.heapify(heap)
    counts = np.zeros(nblocks, dtype=np.int64)
    blk_of = np.empty(N, dtype=np.int64)
    slot_of = np.empty(N, dtype=np.int64)
    for n in order:
        load, b = heapq.heappop(heap)
        blk_of[n] = b
        slot_of[n] = counts[b]
        counts[b] += 1
        load += int(deg[n])
        if counts[b] < P:
            heapq.heappush(heap, (load, b))

    # node -> table row (= global slot id)
    row_of = blk_of * P + slot_of

    eb = blk_of[dst]
    dloc = slot_of[dst]
    ecnt = np.bincount(eb, minlength=nblocks)
    nchunk = max(1, math.ceil((ecnt.max() + 1) / P))  # +1 => >=1 pad slot
    cap = nchunk * P

    order_e = np.argsort(eb, kind="stable")
    starts = np.concatenate([[0], np.cumsum(ecnt)])
    pos = np.arange(Etot, dtype=np.int64) - starts[eb[order_e]]

    # split each block's edges by src table-row range (ucode offset limit);
    # per-range chunk groups, separately padded -> all indices valid + small.
    srow = row_of[src]
    arow_local = (blk_of[dst] % bpc) * P + slot_of[dst]  # dst row in TabAR
    nranges = max(1, math.ceil(slots / RNG))
    rid = srow // RNG
    nch = []
    for r in range(nranges):
        cnt_r = np.bincount(eb[rid == r], minlength=nblocks)
        nch.append(math.ceil(cnt_r.max() / P))
    nchunk = sum(nch)
    cap = nchunk * P
    cbase = np.concatenate([[0], np.cumsum(nch)])  # chunk base per range

    gidx = np.zeros((nblocks, cap), dtype=np.int64)
    aidx = np.zeros((nblocks, cap), dtype=np.int64)
    dlocp = np.full((nblocks, cap), 200.0, dtype=np.float32)
    for r in range(nranges):
        if nch[r] == 0:
            continue
        sel = rid == r
        order_r = np.argsort(eb[sel], kind="stable")
        e_r = np.where(sel)[0][order_r]
        cnt_r = np.bincount(eb[sel], minlength=nblocks)
        s_r = np.concatenate([[0], np.cumsum(cnt_r)])
        pos_r = cbase[r] * P + np.arange(e_r.shape[0]) - s_r[eb[e_r]]
        gidx[eb[e_r], pos_r] = srow[e_r] - r * RNG  # pads stay 0 (local row 0)
        aidx[eb[e_r], pos_r] = arow_local[e_r]
        dlocp[eb[e_r], pos_r] = dloc[e_r].astype(np.float32)

    def per_core_idx(a, n):
        out = np.empty((n_cores, bpc, P, n * 8), dtype=np.int16)
        for c in range(n_cores):
            for b in range(bpc):
                out[c, b] = _idx16(a[c * bpc + b], n, None)
        return out

    idx_arrays = {}
    for r in range(nranges):
        if nch[r]:
            idx_arrays[f"x{r}"] = per_core_idx(
                gidx[:, cbase[r] * P : cbase[r + 1] * P], nch[r])
    idx_arrays["ari"] = per_core_idx(aidx, nchunk)

    dloc_d = np.ascontiguousarray(
        dlocp.reshape(n_cores, bpc, nchunk, P).transpose(0, 1, 3, 2)
    )

    # TabAR build rows: global table row for (core-local block tb, partition p)
    arw = np.empty((n_cores, bpc, P, 1), dtype=np.int32)
    for c in range(n_cores):
        for b in range(bpc):
            arw[c, b, :, 0] = (c * bpc + b) * P + np.arange(P)

    # weights: Wcat [256, 272] = [W.T | B_l | B_r]
    W = np.asarray(W, dtype=np.float32)
    attn_l = np.asarray(attn_l, dtype=np.float32).reshape(H, C)
    attn_r = np.asarray(attn_r, dtype=np.float32).reshape(H, C)
    A_l = np.zeros((HC, H), dtype=np.float32)
    A_r = np.zeros((HC, H), dtype=np.float32)
    for h in range(H):
        A_l[h * C : (h + 1) * C, h] = attn_l[h]
        A_r[h * C : (h + 1) * C, h] = attn_r[h]
    WT = np.ascontiguousarray(W.T)  # [in, hc]
    wcat = np.concatenate([WT, WT @ A_l, WT @ A_r], axis=1)  # [256, 272]
    wcat = np.ascontiguousarray(wcat.reshape(2, P, PSROW)).astype(np.float32)

    # x tiles for phase A: [T, 2, 128(in-lane), 128(node)], permuted so that
    # xp rows come out in table-row order.
    T = slots // P
    x_slot = np.zeros((slots, IN), dtype=np.float32)
    x_slot[row_of] = np.asarray(x, dtype=np.float32)
    xt = np.ascontiguousarray(
        x_slot.reshape(T, P, 2, P).transpose(0, 2, 3, 1), dtype=np.float32)

    iota = np.tile(np.arange(P, dtype=np.float32), (P, 1))  # iota[e, d] = d

    meta = dict(
        N=N, n_cores=n_cores, bpc=bpc, nchunk=nchunk, nch=nch,
        cbase=[int(v) for v in cbase], nranges=nranges,
        T=T, slots=slots, row_of=row_of,
    )
    shared = dict(xt=xt, wcat=wcat, iota=iota)
    per_core = [
        dict(dloc=dloc_d[c], arw=arw[c],
             **{k: v[c] for k, v in idx_arrays.items()})
        for c in range(n_cores)
    ]
    return meta, shared, per_core


# ---------------------------------------------------------------- device IR
def _build_program(meta):
    import concourse.bacc as bacc
    import concourse.bass as bass
    import concourse.tile as tile
    from concourse import mybir

    _install_tile_patch()

    bpc, nchunk, T = meta["bpc"], meta["nchunk"], meta["T"]
    nch, cbase, nranges = meta["nch"], meta["cbase"], meta["nranges"]
    n_cores = meta["n_cores"]
    f32 = mybir.dt.float32
    bf16 = mybir.dt.bfloat16
    i16 = mybir.dt.int16
    Alu = mybir.AluOpType
    Act = mybir.ActivationFunctionType

    mdt = mybir.dt.float32r if USE_F32R else f32
    i32 = mybir.dt.int32

    nc = bacc.Bacc("TRN2", target_bir_lowering=False, debug=False,
                   num_devices=n_cores)
    xt_in = nc.dram_tensor("xt", [T, 2, P, P], mdt, kind="ExternalInput").ap()
    wcat_in = nc.dram_tensor("wcat", [2, P, PSROW], mdt, kind="ExternalInput").ap()
    iota_in = nc.dram_tensor("iota", [P, P], f32, kind="ExternalInput").ap()
    dloc_in = nc.dram_tensor("dloc", [bpc, P, nchunk], f32, kind="ExternalInput").ap()
    arw_in = nc.dram_tensor("arw", [bpc, P, 1], i32, kind="ExternalInput").ap()
    xr_in = {}
    for r in range(nranges):
        if nch[r]:
            xr_in[r] = nc.dram_tensor(f"x{r}", [bpc, P, nch[r] * 8], i16,
                                      kind="ExternalInput").ap()
    ari_in = nc.dram_tensor("ari", [bpc, P, nchunk * 8], i16,
                            kind="ExternalInput").ap()
    out_ex = nc.dram_tensor("out", [bpc * P, HC], f32, kind="ExternalOutput").ap()

    # phase A tile grouping (amortize DMA): largest power of two dividing T, <=8
    G8 = 8
    while T % G8:
        G8 //= 2

    with tile.TileContext(nc) as tc:
        with (
            tc.tile_pool(name="const", bufs=1) as cpool,
            tc.tile_pool(name="dram", bufs=1, space="DRAM") as dpool,
        ):
            table = dpool.tile([T * P, ROW], bf16)
            tabAR = dpool.tile([bpc * P, P], bf16)
            wc0 = cpool.tile([P, PSROW], mdt, tag="wc0")
            wc1 = cpool.tile([P, PSROW], mdt, tag="wc1")
            nc.sync.dma_start(wc0[:], wcat_in[0])
            nc.sync.dma_start(wc1[:], wcat_in[1])
            iota_t = cpool.tile([P, P], f32, tag="iota")
            nc.sync.dma_start(iota_t[:], iota_in[:])

            # ---- phase A: projection table
            with (
                tc.tile_pool(name="pa", bufs=3) as pa,
                tc.tile_pool(name="pa_ps", bufs=4, space="PSUM") as paps,
            ):
                for g in range(T // G8):
                    tiles = slice(g * G8, (g + 1) * G8)
                    ld0 = pa.tile([P, G8, P], mdt, tag="ld0")
                    ld1 = pa.tile([P, G8, P], mdt, tag="ld1")
                    nc.sync.dma_start(
                        ld0[:], xt_in[tiles, 0].rearrange("u p n -> p u n"))
                    nc.sync.dma_start(
                        ld1[:], xt_in[tiles, 1].rearrange("u p n -> p u n"))
                    sbX = pa.tile([P, G8, ROW], bf16, tag="sbX")
                    for u in range(G8):
                        ps = paps.tile([P, PSROW], f32)
                        nc.tensor.matmul(ps[:], lhsT=ld0[:, u, :],
                                         rhs=wc0[:], start=True, stop=False)
                        nc.tensor.matmul(ps[:], lhsT=ld1[:, u, :],
                                         rhs=wc1[:], start=False, stop=True)
                        nc.vector.tensor_copy(sbX[:, u, 0:HC], ps[:, 0:HC])
                        nc.vector.tensor_copy(
                            sbX[:, u, HC : HC + 32].bitcast(f32),
                            ps[:, HC : HC + 16],
                        )
                    dst = table[g * G8 * P : (g + 1) * G8 * P, :].rearrange(
                        "(u p) r -> p u r", p=P
                    )
                    nc.sync.dma_start(dst[:, :, 0 : HC + 32],
                                      sbX[:, :, 0 : HC + 32])

            # ---- phase A': compact per-core [al|ar] table for dst gathers
            with tc.tile_pool(name="par", bufs=3) as par:
                for tb in range(bpc):
                    arw_t = par.tile([P, 1], i32, tag="arw_t")
                    nc.sync.dma_start(arw_t[:], arw_in[tb])
                    rowt = par.tile([P, ROW], bf16, tag="rowt")
                    nc.gpsimd.indirect_dma_start(
                        out=rowt[:], out_offset=None, in_=table[:],
                        in_offset=bass.IndirectOffsetOnAxis(ap=arw_t[:, 0:1],
                                                            axis=0),
                    )
                    nc.sync.dma_start(tabAR[tb * P : (tb + 1) * P, :],
                                      rowt[:, HC : HC + P])

            # ---- phase B: per dst-block gather + attention + scatter
            with (
                tc.tile_pool(name="gat", bufs=3) as gp,
                tc.tile_pool(name="small", bufs=3) as sp,
                tc.tile_pool(name="ps", bufs=2, space="PSUM") as psp,
            ):
                GMAX = 8  # dma_gather tops out at ~1024 indices (128/Q7 core)

                def grouped_gather(src_ap, idx_dram_b, nch, rowe, tag):
                    tiles = []
                    for g0 in range(0, nch, GMAX):
                        gsz = min(GMAX, nch - g0)
                        it = sp.tile([P, gsz * 8], i16, tag=f"{tag}i{g0}",
                                     name=f"{tag}i{g0}")
                        nc.sync.dma_start(it[:],
                                          idx_dram_b[:, g0 * 8 : (g0 + gsz) * 8])
                        gt = gp.tile([P, gsz, rowe], bf16, tag=f"{tag}g{g0}",
                                     name=f"{tag}g{g0}")
                        nc.gpsimd.dma_gather(gt[:], src_ap, it[:], gsz * P,
                                             gsz * P, rowe)
                        tiles.append(gt)
                    return tiles

                for b in range(bpc):
                    dlc = sp.tile([P, nchunk], f32, tag="dlc")
                    nc.sync.dma_start(dlc[:], dloc_in[b])
                    Gr = {}
                    for r in range(nranges):
                        if nch[r]:
                            Gr[r] = grouped_gather(table[r * RNG :, :], xr_in[r][b],
                                                   nch[r], ROW, f"R{r}")
                    Ats = grouped_gather(tabAR[:], ari_in[b], nchunk, P, "A")
                    U = psp.tile([P, HC + H], f32)
                    for j in range(nchunk):
                        r = max(rr for rr in range(nranges)
                                if nch[rr] and cbase[rr] <= j)
                        jj = j - cbase[r]
                        Gj, jj = Gr[r][jj // GMAX], jj % GMAX
                        xpg = Gj[:, jj, 0:HC]
                        al = Gj[:, jj, HC : HC + 32].bitcast(f32)[:, 0:H]
                        Aj = Ats[j // GMAX]
                        ar = Aj[:, j % GMAX, 0:32].bitcast(f32)[:, H : 2 * H]
                        MT = sp.tile([P, HC + H], bf16, tag="MT")
                        lg = sp.tile([P, H], f32, tag="lg")
                        lg2 = sp.tile([P, H], f32, tag="lg2")
                        nc.vector.tensor_tensor(out=lg[:], in0=al, in1=ar,
                                                op=Alu.add)
                        # leaky_relu(x) = max(x, 0.2x), then exp
                        nc.scalar.activation(out=lg2[:], in_=lg[:], func=Act.Copy,
                                             scale=0.2)
                        nc.vector.tensor_tensor(out=lg2[:], in0=lg[:], in1=lg2[:],
                                                op=Alu.max)
                        nc.scalar.activation(out=MT[:, HC : HC + H], in_=lg2[:],
                                             func=Act.Exp)
                        S2 = sp.tile([P, P], bf16, tag="S2")
                        nc.vector.tensor_scalar(S2[:], iota_t[:], dlc[:, j : j + 1],
                                                None, Alu.is_equal)
                        a3 = MT[:, HC : HC + H].unsqueeze(2).to_broadcast([P, H, C])
                        nc.vector.tensor_tensor(
                            out=MT[:, 0:HC].rearrange("p (h c) -> p h c", c=C),
                            in0=xpg.rearrange("p (h c) -> p h c", c=C),
                            in1=a3, op=Alu.mult,
                        )
                        nc.tensor.matmul(U[:], lhsT=S2[:], rhs=MT[:],
                                         start=(j == 0), stop=(j == nchunk - 1))
                    den = sp.tile([P, H], f32, tag="den")
                    nc.vector.tensor_scalar(den[:], U[:, HC : HC + H], 1e-6, None,
                                            Alu.max)
                    rec = sp.tile([P, H], f32, tag="rec")
                    nc.vector.reciprocal(rec[:], den[:])
                    ob = sp.tile([P, HC], f32, tag="ob")
                    r3 = rec[:].unsqueeze(2).to_broadcast([P, H, C])
                    nc.vector.tensor_tensor(
                        out=ob[:].rearrange("p (h c) -> p h c", c=C),
                        in0=U[:, 0:HC].rearrange("p (h c) -> p h c", c=C),
                        in1=r3, op=Alu.mult,
                    )
                    nc.sync.dma_start(out_ex[b * P : (b + 1) * P, :], ob[:])
    nc.compile()
    return nc


# ---------------------------------------------------------------- runner
def _run(inputs, trace=False, n_cores=8):
    from concourse.bass_utils import run_bass_kernel_spmd

    x = np.asarray(inputs["x"])
    edge_index = np.asarray(inputs["edge_index"])
    meta, shared, per_core = _preprocess(
        x, edge_index, inputs["W"], inputs["attn_l"], inputs["attn_r"], n_cores
    )
    nc = _build_program(meta)
    in_maps = [{**shared, **pc} for pc in per_core]
    res = run_bass_kernel_spmd(nc, in_maps, list(range(n_cores)), trace=trace)
    shards = np.concatenate([res.results[c]["out"] for c in range(n_cores)], axis=0)
    out = shards[meta["row_of"]]
    return np.ascontiguousarray(out.astype(np.float32)), res, meta


def kernel(**inputs) -> np.ndarray:
    out, _, _ = _run(inputs, trace=False)
    return out



# revision 3
# speedup vs baseline: 1.7093x; 1.7093x over previous
"""CustomGAT (gnn_message_passing) Trainium2 kernel — 8-core SPMD, v2.

Design (dst-resident, free-axis edge layout):
  * Host: order nodes by (lo-deg, hi-deg) lexsort so consecutive 128-node
    dst blocks have near-equal per-bin in-degrees (few % slot padding).
    Edges of dst p live on partition p along the free axis (j-slots);
    per-edge src table rows split into two int16-addressable bins
    (rows < 32768, base 0; rows >= 32768, base 17408).
  * Device phase A (replicated on all cores): projection table rows
    [xp bf16 x256 | al bf16 x8 | ar bf16 x8 | pad to 768B] via fp32r
    matmuls; one ACT copy per tile PSUM->SBUF. Pad rows 0 and 50001 get
    al=ar=-80 so padded gather slots contribute exp(lrelu(-160+..)) ~ 0.
  * Device phase B (49 dst blocks per core): one m=1 "self" gather call
    brings each dst's own row (= the self-loop edge AND the block's ar
    column); real edges batch-gathered by the GPSIMD dma_gather ucode
    round-robined over 4 SWDGE queues (the descriptor-generation
    bottleneck measured 8.5 ns/row on one queue, ~3.3 ns/row on four).
    Logits al+ar on DVE; exp(leaky_relu(x)) = max(exp(x), exp(0.2x)) via
    two ACT exps + DVE max; messages xp*t on DVE; j-slot reduction via
    identity matmuls accumulating into PSUM [128, 264] whose last 8 cols
    collect the softmax denominator; one normalize per block.
  * Host: inverse-permute per-core shards to the full [N, 256] output.
"""

import numpy as np

# ---------------------------------------------------------------- constants
H = 8
C = 32
HC = H * C  # 256
IN = 256
P = 128
ROW = 384   # bf16 row: [xp 0:256 | al 256:264 | ar 264:272 | pad]
RCOL = 272  # used row columns
PSROW = 272
SLOTS = 50176  # 392 * 128; row 0 = lo pad, rows 50001+ = hi pad area
NBLK = SLOTS // P  # 392
NLO = 256   # blocks 0..255 have rows < 32768
LOWMAX = 32768
HIBASE = 17408  # hi bin rows [32768, 50176) addressed as row - 17408
PADLO = 0
PADHI = 50001
MAXJ = 8  # j-slots per dma_gather call (1024-index ucode cap)
NQ = 4    # SWDGE queues
G8 = 8    # phase A tiles per DMA group


# ---------------------------------------------------------------- tile patch
def _install_tile_patch():
    """The axon-path walrus rejects >2 sync waits on one instruction; split
    the TileContext tail-drain waits into one carrier drain per proc."""
    import concourse.tile as tile
    from concourse.vector_clock import ScopedClock, VectorClock

    if getattr(tile.TileContext, "_drain_patch_installed", False):
        return

    def _drain_and_barrier(self, tick_clock, wait_clock):
        gc = tick_clock.global_clock
        n = len(gc)
        for p in range(n):
            if gc[p] == 0:
                continue
            req = VectorClock([gc[q] if q == p else 0 for q in range(n)])
            d = self.nc.sync.drain()
            wait_clock.add_sem_waits(d.ins, ScopedClock({None: req}))
        self.nc.all_engine_barrier()
        assert self.sems is not None
        popped = self.nc._tile_sem_poison_stack.pop()
        assert popped is self._sem_poison
        self.nc.clear_and_free_semaphores(list(self.sems.allocated().values()))
        self.nc.all_engine_barrier()

    tile.TileContext._drain_and_barrier = _drain_and_barrier
    tile.TileContext._drain_patch_installed = True


# ---------------------------------------------------------------- host prep
def _idx16(vals, m):
    """Encode row indices for dma_gather: [128, m*8] int16, index k at
    [k%16 (+16*rep), k//16], replicated for the 8 Q7 cores."""
    enc = vals.astype(np.int64).astype(np.int16)
    a = enc.reshape(m * 8, 16).T  # [16, m*8]
    return np.tile(a, (8, 1))


def _calls_of(k):
    return [min(MAXJ, k - s) for s in range(0, k, MAXJ)]


def _preprocess(x, edge_index, W, attn_l, attn_r, n_cores):
    import ml_dtypes

    N = x.shape[0]
    src = np.asarray(edge_index[0]).astype(np.int64)
    dst = np.asarray(edge_index[1]).astype(np.int64)

    deg = np.bincount(dst, minlength=N)
    # iterate the (lo-deg, hi-deg) lexsort toward a fixed point
    order = np.argsort(-deg, kind="stable")
    row_of = np.empty(N, np.int64)
    row_of[order] = 1 + np.arange(N)  # row 0 reserved for lo-pad
    for _ in range(3):
        lo = np.bincount(dst[row_of[src] < LOWMAX], minlength=N)
        hi = deg - lo
        order = np.lexsort((hi, lo))
        row_of[order] = 1 + np.arange(N)

    srow = row_of[src]
    drow = row_of[dst]
    ishi = (srow >= LOWMAX).astype(np.int64)

    eblk = drow // P
    epart = drow % P
    cnt = np.zeros((NBLK, P, 2), np.int64)
    np.add.at(cnt, (eblk, epart, ishi), 1)
    klo_b = cnt[:, :, 0].max(axis=1)
    khi_b = cnt[:, :, 1].max(axis=1)
    w_b = klo_b + khi_b

    # positions: 0..31 lo-group (blocks 0..255), 32..48 hi-group
    bpc = NBLK // n_cores  # 49
    npos_lo = NLO // n_cores  # 32
    blk_at = np.empty((bpc, n_cores), np.int64)  # [pos, core] -> block
    for grp, (b0, b1, p0) in enumerate([(0, NLO, 0), (NLO, NBLK, npos_lo)]):
        ranks = b0 + np.argsort(-w_b[b0:b1], kind="stable")
        for i in range((b1 - b0) // n_cores):
            row = ranks[i * n_cores: (i + 1) * n_cores]
            # snake to balance
            blk_at[p0 + i] = row if i % 2 == 0 else row[::-1]

    # uniform per-position call shapes (max over the 8 cores' blocks)
    klo_pos = klo_b[blk_at].max(axis=1)
    khi_pos = khi_b[blk_at].max(axis=1)
    calls_pos = []  # per pos: list of (binflag, m); self call handled apart
    for i in range(bpc):
        calls_pos.append([(0, m) for m in _calls_of(int(klo_pos[i]))]
                         + [(1, m) for m in _calls_of(int(khi_pos[i]))])
    tot_slots = int(sum(1 + klo_pos[i] + khi_pos[i] for i in range(bpc)) * P)

    # per-(block, bin) gather index grids [k, P], padded by bin pad row
    idx_lo = {b: np.full((int(klo_pos[pos]), P), PADLO, np.int64)
              for pos in range(bpc) for b in blk_at[pos]}
    idx_hi = {b: np.full((int(khi_pos[pos]), P), PADHI - HIBASE, np.int64)
              for pos in range(bpc) for b in blk_at[pos]}
    # j index per edge: cumcount within (block, part, bin)
    key = (eblk * P + epart) * 2 + ishi
    eorder = np.argsort(key, kind="stable")
    ks = key[eorder]
    grp_start = np.flatnonzero(np.concatenate([[True], ks[1:] != ks[:-1]]))
    sizes = np.diff(np.concatenate([grp_start, [len(ks)]]))
    jvals = np.arange(len(ks)) - np.repeat(grp_start, sizes)
    ej = np.empty(len(ks), np.int64)
    ej[eorder] = jvals

    lo_m = ishi == 0
    bs, ps, js, rs = eblk[lo_m], epart[lo_m], ej[lo_m], srow[lo_m]
    for b in range(NBLK):
        m = bs == b
        idx_lo[b][js[m], ps[m]] = rs[m]
    hi_m = ishi == 1
    bs, ps, js, rs = eblk[hi_m], epart[hi_m], ej[hi_m], srow[hi_m] - HIBASE
    for b in range(NBLK):
        m = bs == b
        idx_hi[b][js[m], ps[m]] = rs[m]

    # per-core concatenated idx16: [self | lo calls | hi calls] per position
    core_idx = []
    for c in range(n_cores):
        chunks = []
        for pos in range(bpc):
            b = int(blk_at[pos, c])
            base = b * P
            selfidx = np.arange(base, base + P, dtype=np.int64)
            if b >= NLO:
                selfidx = selfidx - HIBASE
            chunks.append(_idx16(selfidx, 1))
            alo, ahi = idx_lo[b], idx_hi[b]
            o = 0
            for m in _calls_of(int(klo_pos[pos])):
                chunks.append(_idx16(alo[o:o + m].reshape(-1), m))
                o += m
            o = 0
            for m in _calls_of(int(khi_pos[pos])):
                chunks.append(_idx16(ahi[o:o + m].reshape(-1), m))
                o += m
        core_idx.append(np.ascontiguousarray(np.concatenate(chunks, axis=1)))
    tot8 = core_idx[0].shape[1]

    # weights: Wcat [256, 272] = [W.T | B_l | B_r]
    W = np.asarray(W, dtype=np.float32)
    attn_l = np.asarray(attn_l, dtype=np.float32).reshape(H, C)
    attn_r = np.asarray(attn_r, dtype=np.float32).reshape(H, C)
    A_l = np.zeros((HC, H), dtype=np.float32)
    A_r = np.zeros((HC, H), dtype=np.float32)
    for h in range(H):
        A_l[h * C: (h + 1) * C, h] = attn_l[h]
        A_r[h * C: (h + 1) * C, h] = attn_r[h]
    WT = np.ascontiguousarray(W.T)
    wcat = np.concatenate([WT, WT @ A_l, WT @ A_r], axis=1)  # [256, 272]
    wcat = np.ascontiguousarray(wcat.reshape(2, P, PSROW)).astype(np.float32)

    # x tiles: [T, 2, 128(in), 128(node)] so xp rows emerge in table order
    T = SLOTS // P
    x_slot = np.zeros((SLOTS, IN), dtype=np.float32)
    x_slot[row_of] = np.asarray(x, dtype=np.float32)
    xt = np.ascontiguousarray(
        x_slot.reshape(T, P, 2, P).transpose(0, 2, 3, 1), dtype=np.float32)

    alpad = np.full((2, 16), -80.0, dtype=ml_dtypes.bfloat16)
    ident = np.eye(P, dtype=ml_dtypes.bfloat16)

    meta = dict(n_cores=n_cores, T=T, bpc=bpc, row_of=row_of, blk_at=blk_at,
                calls_pos=calls_pos, tot8=tot8, tot_slots=tot_slots)
    shared = dict(xt=xt, wcat=wcat, alpad=alpad, ident=ident)
    per_core = [dict(idx=core_idx[c]) for c in range(n_cores)]
    return meta, shared, per_core


# ---------------------------------------------------------------- device IR
def _build_program(meta):
    import concourse.bacc as bacc
    import concourse.tile as tile
    from concourse import mybir

    _install_tile_patch()

    T, bpc, tot8 = meta["T"], meta["bpc"], meta["tot8"]
    calls_pos = meta["calls_pos"]
    n_cores = meta["n_cores"]
    npos_lo = NLO // n_cores
    f32 = mybir.dt.float32
    bf16 = mybir.dt.bfloat16
    i16 = mybir.dt.int16
    f32r = mybir.dt.float32r
    Alu = mybir.AluOpType
    Act = mybir.ActivationFunctionType

    nc = bacc.Bacc("TRN2", target_bir_lowering=False, debug=False,
                   num_devices=n_cores, num_swdge_queues=NQ)
    xt_in = nc.dram_tensor("xt", [T, 2, P, P], f32r, kind="ExternalInput").ap()
    wcat_in = nc.dram_tensor("wcat", [2, P, PSROW], f32r,
                             kind="ExternalInput").ap()
    alpad_in = nc.dram_tensor("alpad", [2, 16], bf16, kind="ExternalInput").ap()
    ident_in = nc.dram_tensor("ident", [P, P], bf16, kind="ExternalInput").ap()
    idx_in = nc.dram_tensor("idx", [P, tot8], i16, kind="ExternalInput").ap()
    out_ex = nc.dram_tensor("out", [bpc * P, HC], bf16,
                            kind="ExternalOutput").ap()

    with tile.TileContext(nc) as tc:
        with (
            tc.tile_pool(name="const", bufs=1) as cpool,
            tc.tile_pool(name="dram", bufs=1, space="DRAM") as dpool,
        ):
            table = dpool.tile([SLOTS, ROW], bf16)
            wc0 = cpool.tile([P, PSROW], f32r, tag="wc0")
            wc1 = cpool.tile([P, PSROW], f32r, tag="wc1")
            nc.sync.dma_start(wc0[:], wcat_in[0])
            nc.sync.dma_start(wc1[:], wcat_in[1])
            idt = cpool.tile([P, P], bf16, tag="idt")
            nc.sync.dma_start(idt[:], ident_in[:])
            alp = cpool.tile([2, 16], bf16, tag="alp")
            nc.sync.dma_start(alp[:], alpad_in[:])
            idx_t = cpool.tile([P, tot8], i16, tag="idx_t")
            nc.sync.dma_start(idx_t[:], idx_in[:])

            # ---- phase A: projection table
            with (
                tc.tile_pool(name="pa", bufs=3) as pa,
                tc.tile_pool(name="pa_ps", bufs=4, space="PSUM") as paps,
            ):
                for g in range(T // G8):
                    tiles = slice(g * G8, (g + 1) * G8)
                    ld0 = pa.tile([P, G8, P], f32r, tag="ld0")
                    ld1 = pa.tile([P, G8, P], f32r, tag="ld1")
                    nc.sync.dma_start(
                        ld0[:], xt_in[tiles, 0].rearrange("u p n -> p u n"))
                    nc.sync.dma_start(
                        ld1[:], xt_in[tiles, 1].rearrange("u p n -> p u n"))
                    sbX = pa.tile([P, G8, RCOL], bf16, tag="sbX")
                    for u in range(G8):
                        ps = paps.tile([P, PSROW], f32)
                        nc.tensor.matmul(ps[:], lhsT=ld0[:, u, :],
                                         rhs=wc0[:], start=True, stop=False)
                        nc.tensor.matmul(ps[:], lhsT=ld1[:, u, :],
                                         rhs=wc1[:], start=False, stop=True)
                        nc.scalar.activation(out=sbX[:, u, :], in_=ps[:],
                                             func=Act.Copy)
                    dst = table[g * G8 * P: (g + 1) * G8 * P, :].rearrange(
                        "(u p) r -> p u r", p=P)
                    nc.sync.dma_start(dst[:, :, 0:RCOL], sbX[:])
            # patch pad-row attention logits to -80
            nc.sync.dma_start(table[PADLO: PADLO + 1, HC: HC + 16],
                              alp[0:1, :])
            nc.sync.dma_start(table[PADHI: PADHI + 1, HC: HC + 16],
                              alp[1:2, :])

            # ---- phase B: per dst-block gather + attention + accumulate
            with (
                tc.tile_pool(name="gat", bufs=8) as gp,
                tc.tile_pool(name="mt", bufs=6) as mp,
                tc.tile_pool(name="small", bufs=6) as sp,
                tc.tile_pool(name="ps", bufs=2, space="PSUM") as psp,
            ):
                qrr = [0]
                off8 = [0]

                def gather(m, hi_base, tag):
                    gt = gp.tile([P, MAXJ, ROW], bf16, tag=tag)
                    src_ap = table[HIBASE:, :] if hi_base else table[:, :]
                    nc.gpsimd.dma_gather(
                        gt[:, 0:m, :], src_ap,
                        idx_t[:, off8[0]: off8[0] + m * 8],
                        m * P, m * P, ROW, queue_num=qrr[0])
                    qrr[0] = (qrr[0] + 1) % NQ
                    off8[0] += m * 8
                    return gt

                for pos in range(bpc):
                    hi_blk = pos >= npos_lo
                    ncalls = 1 + len(calls_pos[pos])
                    U = psp.tile([P, HC + H], f32)
                    gs = gather(1, hi_blk, "G")
                    ar_bc = gs[:, 0:1, HC + H: HC + 2 * H]
                    ci = 0
                    for binf, m in [(None, 1)] + calls_pos[pos]:
                        gt = gs if ci == 0 else gather(m, binf == 1, "G")
                        lg = sp.tile([P, MAXJ, H], f32, tag="lg")
                        nc.vector.tensor_tensor(
                            out=lg[:, 0:m, :],
                            in0=gt[:, 0:m, HC: HC + H],
                            in1=ar_bc.to_broadcast([P, m, H]),
                            op=Alu.add)
                        mt = mp.tile([P, MAXJ, HC + H], bf16, tag="MT")
                        e2 = sp.tile([P, MAXJ, H], bf16, tag="e2")
                        nc.scalar.activation(out=mt[:, 0:m, HC: HC + H],
                                             in_=lg[:, 0:m, :], func=Act.Exp)
                        nc.scalar.activation(out=e2[:, 0:m, :],
                                             in_=lg[:, 0:m, :], func=Act.Exp,
                                             scale=0.2)
                        nc.vector.tensor_tensor(
                            out=mt[:, 0:m, HC: HC + H],
                            in0=mt[:, 0:m, HC: HC + H],
                            in1=e2[:, 0:m, :], op=Alu.max)
                        nc.vector.tensor_tensor(
                            out=mt[:, 0:m, 0:HC].rearrange(
                                "p m (h c) -> p m h c", c=C),
                            in0=gt[:, 0:m, 0:HC].rearrange(
                                "p m (h c) -> p m h c", c=C),
                            in1=mt[:, 0:m, HC: HC + H].unsqueeze(3)
                                .to_broadcast([P, m, H, C]),
                            op=Alu.mult)
                        for j in range(m):
                            nc.tensor.matmul(
                                U[:], lhsT=idt[:], rhs=mt[:, j, :],
                                start=(ci == 0 and j == 0),
                                stop=(ci == ncalls - 1 and j == m - 1))
                        ci += 1
                    den = sp.tile([P, H], f32, tag="den")
                    nc.vector.tensor_scalar(den[:], U[:, HC: HC + H], 1e-6,
                                            None, Alu.max)
                    rec = sp.tile([P, H], f32, tag="rec")
                    nc.vector.reciprocal(rec[:], den[:])
                    ob = sp.tile([P, HC], bf16, tag="ob")
                    nc.vector.tensor_tensor(
                        out=ob[:].rearrange("p (h c) -> p h c", c=C),
                        in0=U[:, 0:HC].rearrange("p (h c) -> p h c", c=C),
                        in1=rec[:].unsqueeze(2).to_broadcast([P, H, C]),
                        op=Alu.mult)
                    nc.sync.dma_start(out_ex[pos * P: (pos + 1) * P, :], ob[:])
    nc.compile()
    return nc


# ---------------------------------------------------------------- runner
def _run(inputs, trace=False, n_cores=8):
    from concourse.bass_utils import run_bass_kernel_spmd

    x = np.asarray(inputs["x"])
    edge_index = np.asarray(inputs["edge_index"])
    meta, shared, per_core = _preprocess(
        x, edge_index, inputs["W"], inputs["attn_l"], inputs["attn_r"], n_cores
    )
    nc = _build_program(meta)
    in_maps = [{**shared, **pc} for pc in per_core]
    res = run_bass_kernel_spmd(nc, in_maps, list(range(n_cores)), trace=trace)

    # reassemble: block at (pos, core) covers table rows [b*128, b*128+128)
    blk_at = meta["blk_at"]
    bpc, row_of = meta["bpc"], meta["row_of"]
    full = np.zeros((SLOTS, HC), np.float32)
    for c in range(n_cores):
        shard = np.asarray(res.results[c]["out"], dtype=np.float32)
        for pos in range(bpc):
            b = int(blk_at[pos, c])
            full[b * P: (b + 1) * P] = shard[pos * P: (pos + 1) * P]
    out = full[row_of]
    return np.ascontiguousarray(out), res, meta


def kernel(**inputs) -> np.ndarray:
    out, _, _ = _run(inputs, trace=False)
    return out
